# revision 6
# baseline (speedup 1.0000x reference)
"""Multi-head attention (B=2, N=2048, D=1024, H=16) on 8 trn2 cores.

Sharding: tensor-parallel over heads — each core computes 2 heads' QKV
projections + attention + its partial W_o product; the host sums the 8
partials (the all-reduce of the W_o row-sharding, done at unshard time).

Per-core DRAM layouts (feature-major / transposed):
  xT   [1024 d_in, 4096 tok]          bf16
  wq/wk/wv [1024 d_in, 128 f]         bf16 (f = 2 heads x 64, head-major)
  wo   [130, 1024]                    bf16 (two 65-row blocks per head:
                                       row 0 zero, rows 1-64 = W_o slice)
  out  [1024 d_out, 4096 tok]         fp32 (partial; host sums over cores)

Attention computes scores transposed ([keys, queries] in PSUM) so the
softmax denominator is a matmul contraction: V carries a leading ones
column (lhsT [128 k, 65] = [ones | V_h]), so the attn@V accumulator's
row 0 is the denominator and rows 1-64 the weighted values. The
normalization multiplies rows 0-64 by a rank-1 broadcast of 1/denom
(row 0 becomes 1, absorbed by the zero row in wo).
"""
import sys

sys.path.insert(0, "/opt/trn_rl_repo")

import numpy as np
import ml_dtypes

B, N, D, H = 2, 2048, 1024, 16
HD = D // H          # 64
N_CORES = 8
HPC = H // N_CORES   # heads per core = 2
F = HPC * HD         # per-core features = 128
T = B * N            # tokens = 4096
TCH = 512            # token chunk (QKV phase, q-window)
KT = 128             # key tile
NKT = N // KT        # 16 key tiles per batch
SCALE = 1.0 / np.sqrt(HD)

_BUILT = None


def _build():
    import concourse.tile as tile
    from concourse import bacc, mybir

    bf16 = mybir.dt.bfloat16
    f32 = mybir.dt.float32

    nc = bacc.Bacc("TRN2", target_bir_lowering=False, debug=False,
                   num_devices=N_CORES)
    xT_d = nc.dram_tensor("xT", [D, T], bf16, kind="ExternalInput").ap()
    wq_d = nc.dram_tensor("wq", [D, F], bf16, kind="ExternalInput").ap()
    wk_d = nc.dram_tensor("wk", [D, F], bf16, kind="ExternalInput").ap()
    wv_d = nc.dram_tensor("wv", [D, F], bf16, kind="ExternalInput").ap()
    wo_d = nc.dram_tensor("wo", [2 * (HD + 1), D], bf16,
                          kind="ExternalInput").ap()
    out_d = nc.dram_tensor("out", [D, T], f32, kind="ExternalOutput").ap()

    DCH = D // 128  # 8 contraction chunks for the projections

    with tile.TileContext(nc) as tc:
        with (
            tc.tile_pool(name="wpool", bufs=1) as wpool,
            tc.tile_pool(name="xin", bufs=16) as xin,
            tc.tile_pool(name="qkv", bufs=1) as qkv,
            tc.tile_pool(name="expp", bufs=3) as expp,
            tc.tile_pool(name="small", bufs=4) as small,
            tc.tile_pool(name="ps", bufs=2, space="PSUM") as ps,
        ):
            # --- weights to SBUF ---
            wq_sb = wpool.tile([128, D], bf16, tag="wq")   # [p, dchunk*128+f]
            wk_sb = wpool.tile([128, D], bf16, tag="wk")
            wv_sb = wpool.tile([128, D], bf16, tag="wv")
            wo_sb = [wpool.tile([HD + 1, D], bf16, tag=f"wo{h}", name=f"wo{h}")
                     for h in range(HPC)]
            for w_sb, w_d in ((wq_sb, wq_d), (wk_sb, wk_d), (wv_sb, wv_d)):
                nc.sync.dma_start(
                    w_sb.rearrange("p (c f) -> p c f", f=F),
                    w_d.rearrange("(c p) f -> p c f", p=128))
            for h in range(HPC):
                nc.sync.dma_start(
                    wo_sb[h][:], wo_d[h * (HD + 1):(h + 1) * (HD + 1), :])
            ones_sb = wpool.tile([1, HD + 1], f32, tag="ones")
            nc.vector.memset(ones_sb[:], 1.0)

            # --- per-batch persistent tensors ---
            q_sb = [qkv.tile([128, N], bf16, tag=f"q{b}", name=f"q{b}")
                    for b in range(B)]
            k_sb = [qkv.tile([128, N], bf16, tag=f"k{b}", name=f"k{b}")
                    for b in range(B)]
            # v: per batch [128 tok, 16 ktiles x 2 heads x (ones|64 vals)]
            VW = NKT * 2 * (HD + 1)  # 2080
            v_sb = [qkv.tile([128, VW], bf16, tag=f"v{b}", name=f"v{b}")
                    for b in range(B)]
            # concat, per (batch, head): row 0 garbage, rows 1-64 data
            cc_sb = [[qkv.tile([HD + 1, N], bf16, tag=f"c{b}{h}",
                               name=f"c{b}{h}")
                      for h in range(HPC)] for b in range(B)]

            for b in range(B):
                nc.vector.memset(
                    v_sb[b].rearrange("p (g c) -> p g c", c=HD + 1)[:, :, 0:1],
                    1.0)

            # --- QKV projections, batch-major so attention(b0) can start
            # while QKV(b1) runs ---
            for b in range(B):
                for wc in range(N // TCH):  # 4 chunks of 512 tokens
                    toff = b * N + wc * TCH
                    xt = [xin.tile([128, TCH], bf16, tag="x",
                                   name=f"x{b}_{wc}_{c}")
                          for c in range(DCH)]
                    for c in range(DCH):
                        nc.sync.dma_start(
                            xt[c][:], xT_d[c * 128:(c + 1) * 128,
                                           toff:toff + TCH])
                    # Q, K: accumulate over d chunks -> [128 f, 512 tok]
                    for w_sb, dst in ((wq_sb, q_sb[b]), (wk_sb, k_sb[b])):
                        acc = ps.tile([128, 1024], f32, tag="big")
                        for c in range(DCH):
                            nc.tensor.matmul(
                                acc[:, 0:TCH],
                                w_sb[:, c * 128:(c + 1) * 128],
                                xt[c][:],
                                start=(c == 0), stop=(c == DCH - 1))
                        nc.vector.tensor_copy(
                            dst[:, wc * TCH:(wc + 1) * TCH], acc[:, 0:TCH])
                    # V: [tok, f] layout, per 128-token tile
                    for tt4 in range(TCH // 128):
                        tt = wc * 4 + tt4  # tok-tile index within batch
                        acc = ps.tile([128, 1024], f32, tag="big")
                        for c in range(DCH):
                            nc.tensor.matmul(
                                acc[:, 0:128],
                                xt[c][:, tt4 * 128:(tt4 + 1) * 128],
                                wv_sb[:, c * 128:(c + 1) * 128],
                                start=(c == 0), stop=(c == DCH - 1))
                        for h in range(HPC):
                            g = tt * 2 * (HD + 1) + h * (HD + 1)
                            nc.vector.tensor_copy(
                                v_sb[b][:, g + 1:g + HD + 1],
                                acc[:, h * HD:(h + 1) * HD])

                # --- attention for batch b ---
                for wc in range(N // TCH):  # 4 query windows of 512
                    qoff = wc * TCH
                    av = [ps.tile([HD + 1, TCH], f32, tag="av",
                                  name=f"av{b}_{wc}_{h}")
                          for h in range(HPC)]
                    for kt in range(NKT):
                        koff = kt * KT
                        sc = ps.tile([128, 1024], f32, tag="big")
                        for h in range(HPC):
                            nc.tensor.matmul(
                                sc[:, h * TCH:(h + 1) * TCH],
                                k_sb[b][h * HD:(h + 1) * HD, koff:koff + KT],
                                q_sb[b][h * HD:(h + 1) * HD, qoff:qoff + TCH],
                                start=True, stop=True)
                        ex = expp.tile([128, 1024], bf16, tag="e")
                        nc.scalar.activation(
                            ex[:], sc[:], mybir.ActivationFunctionType.Exp,
                            scale=float(SCALE))
                        for h in range(HPC):
                            g = kt * 2 * (HD + 1) + h * (HD + 1)
                            nc.tensor.matmul(
                                av[h][:],
                                v_sb[b][:, g:g + HD + 1],
                                ex[:, h * TCH:(h + 1) * TCH],
                                start=(kt == 0), stop=(kt == NKT - 1))
                    # normalize: cc rows 0-64 = av * (1/av[0]) broadcast
                    for h in range(HPC):
                        rec = small.tile([1, TCH], f32, tag="rec")
                        nc.vector.reciprocal(rec[:], av[h][0:1, :])
                        bc = ps.tile([HD + 1, TCH], f32, tag="wo")
                        nc.tensor.matmul(bc[:], ones_sb[:], rec[:],
                                         start=True, stop=True)
                        bcs = small.tile([HD + 1, TCH], f32, tag="bcs")
                        nc.vector.tensor_copy(bcs[:], bc[:])
                        nc.vector.tensor_mul(
                            cc_sb[b][h][:, qoff:qoff + TCH], av[h][:], bcs[:])
                    # partial W_o for this token window (K=65 per head,
                    # zero weight row absorbs cc's garbage row 0)
                    for j in range(DCH):
                        acc = ps.tile([128, TCH], f32, tag="wo")
                        for h in range(HPC):
                            nc.tensor.matmul(
                                acc[:],
                                wo_sb[h][:, j * 128:(j + 1) * 128],
                                cc_sb[b][h][:, qoff:qoff + TCH],
                                start=(h == 0), stop=(h == HPC - 1))
                        od = small.tile([128, TCH], f32, tag="od",
                                        name=f"od{b}_{wc}_{j}")
                        nc.vector.tensor_copy(od[:], acc[:])
                        nc.sync.dma_start(
                            out_d[j * 128:(j + 1) * 128,
                                  b * N + qoff:b * N + qoff + TCH],
                            od[:])
    nc.compile()
    return nc


def _get_built():
    global _BUILT
    if _BUILT is None:
        _BUILT = _build()
    return _BUILT


def _head_rows(h):
    # feature d of head h sits at column i*H + h of the QKV projection
    # output (einops 'b n (d h)' with head as the inner factor)
    return np.arange(HD) * H + h


def kernel(x, W_q, W_k, W_v, W_o):
    from concourse.bass_utils import run_bass_kernel_spmd

    nc = _get_built()
    bf = ml_dtypes.bfloat16
    xT = np.ascontiguousarray(
        np.asarray(x, dtype=np.float32).reshape(T, D).T).astype(bf)
    W_q, W_k, W_v, W_o = (np.asarray(w, dtype=np.float32)
                          for w in (W_q, W_k, W_v, W_o))
    in_maps = []
    for c in range(N_CORES):
        rows = np.concatenate([_head_rows(2 * c), _head_rows(2 * c + 1)])
        wo_blocks = []
        for h in (2 * c, 2 * c + 1):
            # concat ('b h n d -> b n (h d)') puts head h's features at
            # columns [h*64, (h+1)*64)
            blk = W_o[:, h * HD:(h + 1) * HD].T  # [64, 1024]
            wo_blocks.append(np.zeros((1, D), np.float32))
            wo_blocks.append(blk)
        in_maps.append({
            "xT": xT,
            "wq": np.ascontiguousarray(W_q[rows, :].T).astype(bf),
            "wk": np.ascontiguousarray(W_k[rows, :].T).astype(bf),
            "wv": np.ascontiguousarray(W_v[rows, :].T).astype(bf),
            "wo": np.ascontiguousarray(
                np.concatenate(wo_blocks, axis=0)).astype(bf),
        })
    res = run_bass_kernel_spmd(nc, in_maps, list(range(N_CORES)))
    total = np.zeros((D, T), dtype=np.float32)
    for c in range(N_CORES):
        total += res.results[c]["out"]
    return np.ascontiguousarray(total.T).reshape(B, N, D)


# revision 11
# speedup vs baseline: 26.7166x; 26.7166x over previous
"""Multi-head attention (B=2, N=2048, D=1024, H=16) on 8 trn2 cores.

Sharding: tensor-parallel over heads — each core computes 2 heads' QKV
projections + attention + its partial W_o product; the host sums the 8
partials (the all-reduce of the W_o row-sharding, done at unshard time).

Per-core DRAM layouts (feature-major / transposed):
  xT   [1024 d_in, 4096 tok]          bf16
  wq/wk/wv [1024 d_in, 128 f]         bf16 (f = 2 heads x 64, head-major)
  wo   [130, 1024]                    bf16 (two 65-row blocks per head:
                                       row 0 zero, rows 1-64 = W_o slice)
  out  [1024 d_out, 4096 tok]         fp32 (partial; host sums over cores)

Attention computes scores transposed ([keys, queries] in PSUM) so the
softmax denominator is a matmul contraction: V carries a leading ones
column (lhsT [128 k, 65] = [ones | V_h]), so the attn@V accumulator's
row 0 is the denominator and rows 1-64 the weighted values. The
normalization multiplies rows 0-64 by a rank-1 broadcast of 1/denom
(row 0 becomes 1, absorbed by the zero row in wo).
"""
import sys

sys.path.insert(0, "/opt/trn_rl_repo")

import numpy as np
import ml_dtypes

B, N, D, H = 2, 2048, 1024, 16
HD = D // H          # 64
N_CORES = 8
HPC = H // N_CORES   # heads per core = 2
F = HPC * HD         # per-core features = 128
T = B * N            # tokens = 4096
TCH = 512            # token chunk (QKV phase, q-window)
KT = 128             # key tile
NKT = N // KT        # 16 key tiles per batch
SCALE = 1.0 / np.sqrt(HD)

_BUILT = None


def _build():
    import concourse.tile as tile
    from concourse import bacc, mybir

    bf16 = mybir.dt.bfloat16
    f32 = mybir.dt.float32

    nc = bacc.Bacc("TRN2", target_bir_lowering=False, debug=False,
                   num_devices=N_CORES)
    xT_d = nc.dram_tensor("xT", [D, T], bf16, kind="ExternalInput").ap()
    wq_d = nc.dram_tensor("wq", [D, F], bf16, kind="ExternalInput").ap()
    wk_d = nc.dram_tensor("wk", [D, F], bf16, kind="ExternalInput").ap()
    wv_d = nc.dram_tensor("wv", [D, F], bf16, kind="ExternalInput").ap()
    wo_d = nc.dram_tensor("wo", [2 * (HD + 1), D], bf16,
                          kind="ExternalInput").ap()
    out_d = nc.dram_tensor("out", [D, T], f32, kind="ExternalOutput").ap()

    DCH = D // 128  # 8 contraction chunks for the projections

    with tile.TileContext(nc) as tc:
        with (
            tc.tile_pool(name="wpool", bufs=1) as wpool,
            tc.tile_pool(name="xin", bufs=16) as xin,
            tc.tile_pool(name="qkv", bufs=1) as qkv,
            tc.tile_pool(name="expp", bufs=3) as expp,
            tc.tile_pool(name="small", bufs=4) as small,
            tc.tile_pool(name="ps", bufs=2, space="PSUM") as ps,
        ):
            # --- weights to SBUF ---
            wq_sb = wpool.tile([128, D], bf16, tag="wq")   # [p, dchunk*128+f]
            wk_sb = wpool.tile([128, D], bf16, tag="wk")
            wv_sb = wpool.tile([128, D], bf16, tag="wv")
            wo_sb = [wpool.tile([HD + 1, D], bf16, tag=f"wo{h}", name=f"wo{h}")
                     for h in range(HPC)]
            for w_sb, w_d in ((wq_sb, wq_d), (wk_sb, wk_d), (wv_sb, wv_d)):
                nc.sync.dma_start(
                    w_sb.rearrange("p (c f) -> p c f", f=F),
                    w_d.rearrange("(c p) f -> p c f", p=128))
            for h in range(HPC):
                nc.sync.dma_start(
                    wo_sb[h][:], wo_d[h * (HD + 1):(h + 1) * (HD + 1), :])
            ones_sb = wpool.tile([1, HD + 1], f32, tag="ones")
            nc.vector.memset(ones_sb[:], 1.0)

            # --- per-batch persistent tensors ---
            q_sb = [qkv.tile([128, N], bf16, tag=f"q{b}", name=f"q{b}")
                    for b in range(B)]
            k_sb = [qkv.tile([128, N], bf16, tag=f"k{b}", name=f"k{b}")
                    for b in range(B)]
            # v: per batch [128 tok, 16 ktiles x 2 heads x (ones|64 vals)]
            VW = NKT * 2 * (HD + 1)  # 2080
            v_sb = [qkv.tile([128, VW], bf16, tag=f"v{b}", name=f"v{b}")
                    for b in range(B)]
            # concat, per (batch, head): row 0 garbage, rows 1-64 data
            cc_sb = [[qkv.tile([HD + 1, N], bf16, tag=f"c{b}{h}",
                               name=f"c{b}{h}")
                      for h in range(HPC)] for b in range(B)]

            for b in range(B):
                nc.vector.memset(
                    v_sb[b].rearrange("p (g c) -> p g c", c=HD + 1)[:, :, 0:1],
                    1.0)

            # --- QKV projections, batch-major so attention(b0) can start
            # while QKV(b1) runs ---
            for b in range(B):
                for wc in range(N // TCH):  # 4 chunks of 512 tokens
                    toff = b * N + wc * TCH
                    xt = [xin.tile([128, TCH], bf16, tag="x",
                                   name=f"x{b}_{wc}_{c}")
                          for c in range(DCH)]
                    for c in range(DCH):
                        nc.sync.dma_start(
                            xt[c][:], xT_d[c * 128:(c + 1) * 128,
                                           toff:toff + TCH])
                    # Q, K: accumulate over d chunks -> [128 f, 512 tok]
                    for w_sb, dst in ((wq_sb, q_sb[b]), (wk_sb, k_sb[b])):
                        acc = ps.tile([128, 1024], f32, tag="big")
                        for c in range(DCH):
                            nc.tensor.matmul(
                                acc[:, 0:TCH],
                                w_sb[:, c * 128:(c + 1) * 128],
                                xt[c][:],
                                start=(c == 0), stop=(c == DCH - 1))
                        nc.vector.tensor_copy(
                            dst[:, wc * TCH:(wc + 1) * TCH], acc[:, 0:TCH])
                    # V: [tok, f] layout, per 128-token tile
                    for tt4 in range(TCH // 128):
                        tt = wc * 4 + tt4  # tok-tile index within batch
                        acc = ps.tile([128, 1024], f32, tag="big")
                        for c in range(DCH):
                            nc.tensor.matmul(
                                acc[:, 0:128],
                                xt[c][:, tt4 * 128:(tt4 + 1) * 128],
                                wv_sb[:, c * 128:(c + 1) * 128],
                                start=(c == 0), stop=(c == DCH - 1))
                        for h in range(HPC):
                            g = tt * 2 * (HD + 1) + h * (HD + 1)
                            nc.vector.tensor_copy(
                                v_sb[b][:, g + 1:g + HD + 1],
                                acc[:, h * HD:(h + 1) * HD])

                # --- attention for batch b ---
                for wc in range(N // TCH):  # 4 query windows of 512
                    qoff = wc * TCH
                    av = [ps.tile([HD + 1, TCH], f32, tag="av",
                                  name=f"av{b}_{wc}_{h}")
                          for h in range(HPC)]
                    for kt in range(NKT):
                        koff = kt * KT
                        sc = ps.tile([128, 1024], f32, tag="big")
                        for h in range(HPC):
                            nc.tensor.matmul(
                                sc[:, h * TCH:(h + 1) * TCH],
                                k_sb[b][h * HD:(h + 1) * HD, koff:koff + KT],
                                q_sb[b][h * HD:(h + 1) * HD, qoff:qoff + TCH],
                                start=True, stop=True)
                        ex = expp.tile([128, 1024], bf16, tag="e")
                        nc.scalar.activation(
                            ex[:], sc[:], mybir.ActivationFunctionType.Exp,
                            scale=float(SCALE))
                        for h in range(HPC):
                            g = kt * 2 * (HD + 1) + h * (HD + 1)
                            nc.tensor.matmul(
                                av[h][:],
                                v_sb[b][:, g:g + HD + 1],
                                ex[:, h * TCH:(h + 1) * TCH],
                                start=(kt == 0), stop=(kt == NKT - 1))
                    # normalize: cc rows 0-64 = av * (1/av[0]) broadcast
                    for h in range(HPC):
                        rec = small.tile([1, TCH], f32, tag="rec")
                        nc.vector.reciprocal(rec[:], av[h][0:1, :])
                        bc = ps.tile([HD + 1, TCH], f32, tag="wo")
                        nc.tensor.matmul(bc[:], ones_sb[:], rec[:],
                                         start=True, stop=True)
                        bcs = small.tile([HD + 1, TCH], f32, tag="bcs")
                        nc.vector.tensor_copy(bcs[:], bc[:])
                        nc.vector.tensor_mul(
                            cc_sb[b][h][:, qoff:qoff + TCH], av[h][:], bcs[:])
                    # partial W_o for this token window (K=65 per head,
                    # zero weight row absorbs cc's garbage row 0)
                    for j in range(DCH):
                        acc = ps.tile([128, TCH], f32, tag="wo")
                        for h in range(HPC):
                            nc.tensor.matmul(
                                acc[:],
                                wo_sb[h][:, j * 128:(j + 1) * 128],
                                cc_sb[b][h][:, qoff:qoff + TCH],
                                start=(h == 0), stop=(h == HPC - 1))
                        od = small.tile([128, TCH], f32, tag="od",
                                        name=f"od{b}_{wc}_{j}")
                        nc.vector.tensor_copy(od[:], acc[:])
                        nc.sync.dma_start(
                            out_d[j * 128:(j + 1) * 128,
                                  b * N + qoff:b * N + qoff + TCH],
                            od[:])
    nc.compile()
    return nc


def _get_built():
    global _BUILT
    if _BUILT is None:
        _BUILT = _build()
    return _BUILT


_RUNNER = None


def _get_runner():
    """Build the sharded jitted executable once (jax.jit caches on function
    identity, so run_bass_kernel_spmd would re-trace every call)."""
    global _RUNNER
    if _RUNNER is not None:
        return _RUNNER
    import jax
    import numpy as _np
    from jax.sharding import Mesh, PartitionSpec
    from jax.experimental.shard_map import shard_map
    from concourse import mybir
    from concourse.bass2jax import (
        install_neuronx_cc_hook, _bass_exec_p, partition_id_tensor)

    nc = _get_built()
    install_neuronx_cc_hook()

    partition_name = (nc.partition_id_tensor.name
                      if nc.partition_id_tensor else None)
    in_names, out_names, out_avals, zero_shapes = [], [], [], []
    for alloc in nc.m.functions[0].allocations:
        if not isinstance(alloc, mybir.MemoryLocationSet):
            continue
        name = alloc.memorylocations[0].name
        if alloc.kind == "ExternalInput":
            if name != partition_name:
                in_names.append(name)
        elif alloc.kind == "ExternalOutput":
            np_dt = mybir.dt.np(alloc.dtype)
            out_avals.append(
                jax.core.ShapedArray(tuple(alloc.tensor_shape), np_dt))
            out_names.append(name)
            zero_shapes.append((tuple(alloc.tensor_shape), np_dt))
    n_params = len(in_names)
    all_names = in_names + out_names
    if partition_name is not None:
        all_names = all_names + [partition_name]

    def _body(*args):
        operands = list(args)
        if partition_name is not None:
            operands.append(partition_id_tensor())
        outs = _bass_exec_p.bind(
            *operands,
            out_avals=tuple(out_avals),
            in_names=tuple(all_names),
            out_names=tuple(out_names),
            lowering_input_output_aliases=(),
            sim_require_finite=True,
            sim_require_nnan=True,
            nc=nc,
        )
        return tuple(outs)

    devices = jax.devices()[:N_CORES]
    mesh = Mesh(_np.asarray(devices), ("core",))
    n_outs = len(out_names)
    sharded = jax.jit(
        shard_map(_body, mesh=mesh,
                  in_specs=(PartitionSpec("core"),) * (n_params + n_outs),
                  out_specs=(PartitionSpec("core"),) * n_outs,
                  check_rep=False),
        donate_argnums=tuple(range(n_params, n_params + n_outs)),
        keep_unused=True,
    )

    def run(in_maps):
        concat_in = [
            _np.concatenate([in_maps[c][nm] for c in range(N_CORES)], axis=0)
            for nm in in_names]
        concat_zeros = [
            _np.zeros((N_CORES * s[0], *s[1:]), dt) for s, dt in zero_shapes]
        outs = sharded(*concat_in, *concat_zeros)
        return [
            {nm: _np.asarray(outs[i]).reshape(N_CORES, *zero_shapes[i][0])[c]
             for i, nm in enumerate(out_names)}
            for c in range(N_CORES)]

    _RUNNER = run
    return run


def _head_rows(h):
    # feature d of head h sits at column i*H + h of the QKV projection
    # output (einops 'b n (d h)' with head as the inner factor)
    return np.arange(HD) * H + h


def kernel(x, W_q, W_k, W_v, W_o):
    run = _get_runner()
    bf = ml_dtypes.bfloat16
    xT = np.ascontiguousarray(
        np.asarray(x, dtype=np.float32).reshape(T, D).T).astype(bf)
    W_q, W_k, W_v, W_o = (np.asarray(w, dtype=np.float32)
                          for w in (W_q, W_k, W_v, W_o))
    in_maps = []
    for c in range(N_CORES):
        rows = np.concatenate([_head_rows(2 * c), _head_rows(2 * c + 1)])
        wo_blocks = []
        for h in (2 * c, 2 * c + 1):
            # concat ('b h n d -> b n (h d)') puts head h's features at
            # columns [h*64, (h+1)*64)
            blk = W_o[:, h * HD:(h + 1) * HD].T  # [64, 1024]
            wo_blocks.append(np.zeros((1, D), np.float32))
            wo_blocks.append(blk)
        in_maps.append({
            "xT": xT,
            "wq": np.ascontiguousarray(W_q[rows, :].T).astype(bf),
            "wk": np.ascontiguousarray(W_k[rows, :].T).astype(bf),
            "wv": np.ascontiguousarray(W_v[rows, :].T).astype(bf),
            "wo": np.ascontiguousarray(
                np.concatenate(wo_blocks, axis=0)).astype(bf),
        })
    results = run(in_maps)
    total = np.zeros((D, T), dtype=np.float32)
    for c in range(N_CORES):
        total += results[c]["out"]
    return np.ascontiguousarray(total.T).reshape(B, N, D)
